# revision 14
# baseline (speedup 1.0000x reference)
"""Trainium2 Bass kernel for nn_CustomLoss_35940286333129.

loss[b] = mean|pred-target| (mae, scalar)
        + mean(min_n cdist[b,n,m]) + mean(min_b cdist[b,n,m])  (chamfer, scalar)
        + mean|sort(pred[b].ravel()) - sort(target[b].ravel())|  (emd, per-b)

Sharding: data-parallel over batch B=32 across 8 NeuronCores (4 samples each).

Device kernel (per local sample b):
  One fp8 DoubleRow matmul per 128-row tile computes the COMPLETE squared
  distance d2[m, n] = tn[m] + pn[n] - 2*T[m].P[n] directly in PSUM:
  the K=256 contraction carries -2*T^t x P^t in the first K-half and the
  norm biases in the second K-half (tn/pn shipped from the host as 3-term
  fp8 residual cascades against ones rows). 512 PE cycles per tile;
  no ones-matmul, no cast/transpose chains, no DRAM bounce.

  One fused custom DVE op consumes each PSUM tile in a single 1x pass:
      out = where(Idx == 1023, running_min(d2), min(d2, acc))
  so cols 0..1022 update the cross-sample elementwise min (chamfer min over
  dim=0) while col 1023 captures min_n d2 (chamfer min over dim=1), which
  ACT harvests per sample before the next overwrite. PSUM holds four exact
  [128,1024] tiles (no pad column), double-buffering the PE four deep.

Host: fp8 operand packing (transpose/cast/norm cascades) during sharding,
cross-core elementwise min + sqrt + means, the exact column n=1023 of the
chamfer dim-0 min (overwritten on-device by the scan output; 32x1024 dot
products in numpy), mae, and the exact per-sample EMD via np.sort (sort is
unsupported on trn2).
"""

import numpy as np
import ml_dtypes

F8 = ml_dtypes.float8_e4m3

B, N, D = 32, 1024, 128
NCORES = 8
BL = B // NCORES          # 4 local samples per core
NT = N // 128             # 8 row tiles

_CACHE = {}


def _register_ops():
    from concourse import dve_ops
    from concourse.dve_ops import DveOp, OPS, DveOpSpec
    from concourse.dve_spec import (Spec, Src0, Src1, C0, C1, scan, minn,
                                    select, eq, lower, AluOp, Idx)

    def _mk(name, body, ref, rd1):
        for op in OPS:
            if op.name == name:
                return op
        spec = Spec(body=body, reference=ref)
        shas = {}
        for ver in ("v3", "v4"):
            tmp = DveOpSpec(name=name, opcode=0, uops=lower(spec, ver=ver),
                            rd1_en=rd1)
            shas[ver] = tmp.sha(ver)
        op = DveOp(name, spec, subdim=False, uops_sha=shas)
        OPS.append(op)
        dve_ops.CUSTOM_DVE_SPECS[op.name] = op.spec
        dve_ops._SUB_OPCODE_FOR_NAME[op.name] = (
            dve_ops._CUSTOM_DVE_ROW_BASE + len(OPS) - 1)
        return op

    r = scan(AluOp.MIN, Src0, init=C0)

    def ref_acc(in0, in1, s0, s1, imm2):
        idx = np.arange(in0.shape[-1])
        state = np.minimum.accumulate(np.minimum(in0, s0), axis=-1)
        return np.where(idx == s1, state, np.minimum(in0, in1))

    def ref_init(in0, s0, s1, imm2):
        idx = np.arange(in0.shape[-1])
        state = np.minimum.accumulate(np.minimum(in0, s0), axis=-1)
        return np.where(idx == s1, state, in0)

    acc_op = _mk("MINACC_IDX", select(eq(Idx, C1), r, minn(Src0, Src1)),
                 ref_acc, True)
    init_op = _mk("MININIT_IDX", select(eq(Idx, C1), r, Src0), ref_init, False)
    return acc_op, init_op


def _build():
    import concourse.bass as bass
    import concourse.bacc as bacc
    import concourse.tile as tile
    from concourse import mybir

    MINACC, MININIT = _register_ops()

    f32, f16, f8 = mybir.dt.float32, mybir.dt.float16, mybir.dt.float8e4
    AF = mybir.ActivationFunctionType
    DR = mybir.MatmulPerfMode.DoubleRow

    nc = bacc.Bacc("TRN2", target_bir_lowering=False, debug=False,
                   num_devices=NCORES)
    stat_d = nc.declare_dram_parameter("stat8", [BL, 128, NT, 2, 128], f8,
                                       isOutput=False)
    mov_d = nc.declare_dram_parameter("mov8", [BL, 128, 2, N], f8,
                                      isOutput=False)
    ch0_o = nc.declare_dram_parameter("ch0_part", [N, N], f16, isOutput=True)
    ch1_o = nc.declare_dram_parameter("ch1_part", [128, BL, NT], f16,
                                      isOutput=True)

    with tile.TileContext(nc) as tc:
        with (
            tc.tile_pool(name="stat", bufs=2) as statp,
            tc.tile_pool(name="mov", bufs=2) as movp,
            tc.tile_pool(name="persist", bufs=1) as perp,
            tc.tile_pool(name="nps", bufs=1, space=bass.MemorySpace.PSUM) as nps,
        ):
            acc = perp.tile([128, NT, N], f16, tag="acc")
            ch1z = perp.tile([128, BL, NT], f16, tag="ch1z")

            gt = [nps.tile([128, N], f32, tag=f"g{i}", name=f"g{i}")
                  for i in range(4)]

            for b in range(BL):
                # split loads so the first tiles' operands land early; b=0
                # fans out over four DMA queues to shorten the pipeline fill
                stat = statp.tile([128, NT, 2, 128], f8, tag="stat")
                mov = movp.tile([128, 2, N], f8, tag="mov")
                if b == 0:
                    nc.gpsimd.dma_start(stat[:, 0:2], stat_d[b, :, 0:2])
                    nc.scalar.dma_start(mov[:, :, 0:512],
                                        mov_d[b, :, :, 0:512])
                    nc.sync.dma_start(mov[:, :, 512:N],
                                      mov_d[b, :, :, 512:N])
                    nc.sync.dma_start(stat[:, 2:NT], stat_d[b, :, 2:NT])
                else:
                    nc.sync.dma_start(stat[:, 0:2], stat_d[b, :, 0:2])
                    nc.scalar.dma_start(mov[:, :, 0:512],
                                        mov_d[b, :, :, 0:512])
                    nc.sync.dma_start(stat[:, 2:NT], stat_d[b, :, 2:NT])
                    nc.scalar.dma_start(mov[:, :, 512:N],
                                        mov_d[b, :, :, 512:N])

                for mt in range(NT):
                    g = gt[mt % 4]
                    for c in range(2):
                        nc.tensor.matmul(
                            g[:, c * 512:(c + 1) * 512],
                            stat[:, mt, :, :],
                            mov[:, :, c * 512:(c + 1) * 512],
                            start=True, stop=True, perf_mode=DR)
                    if b == 0:
                        nc.vector._custom_dve(
                            MININIT, out=acc[:, mt, :], in0=g[:],
                            s0=60000.0, s1=1023.0)
                    else:
                        nc.vector._custom_dve(
                            MINACC, out=acc[:, mt, :], in0=g[:],
                            in1=acc[:, mt, :], s0=60000.0, s1=1023.0)
                    if b == BL - 1:
                        # acc[mt] final: stream it out under remaining compute,
                        # rotating over all three DMA queues; the last tiles
                        # split so no single transfer tails past the end
                        lo, hi = mt * 128, (mt + 1) * 128
                        rings = [nc.scalar, nc.sync, nc.gpsimd]
                        if mt < NT - 2:
                            rings[mt % 3].dma_start(ch0_o[lo:hi, :],
                                                    acc[:, mt, :])
                        else:
                            r0, r1 = rings[mt % 3], rings[(mt + 1) % 3]
                            r0.dma_start(ch0_o[lo:hi, 0:512],
                                         acc[:, mt, 0:512])
                            r1.dma_start(ch0_o[lo:hi, 512:N],
                                         acc[:, mt, 512:N])
                # harvest this b's min_n d2 (scan cols) before b+1 overwrites;
                # two halves, so half 1 is done before b+1's first custom op
                nc.scalar.activation(out=ch1z[:, b, 0:4],
                                     in_=acc[:, 0:4, N - 1], func=AF.Copy)
                nc.scalar.activation(out=ch1z[:, b, 4:NT],
                                     in_=acc[:, 4:NT, N - 1], func=AF.Copy)
                nc.gpsimd.dma_start(ch1_o[:, b, :], ch1z[:, b, :])

    nc.compile()
    return nc


def _get_nc():
    if "nc" not in _CACHE:
        _CACHE["nc"] = _build()
    return _CACHE["nc"]


def _pack_core(pred_s, targ_s):
    """Build stat8/mov8 fp8 operands for one core's BL samples."""
    stat8 = np.zeros((BL, 128, NT, 2, 128), F8)
    mov8 = np.zeros((BL, 128, 2, N), F8)
    one8 = np.asarray(1.0, F8)
    for b in range(BL):
        T = targ_s[b]                    # [N, D]
        P = pred_s[b]
        tn = (T.astype(np.float64) ** 2).sum(-1).astype(np.float32)  # [N]
        pn = (P.astype(np.float64) ** 2).sum(-1).astype(np.float32)

        # 3-term fp8 residual cascades of tn / pn
        def casc(v):
            terms, rem = [], v.copy()
            for _ in range(3):
                t = np.asarray(rem, F8)
                terms.append(t)
                rem = rem - t.astype(np.float32)
            return terms

        tn_t, pn_t = casc(tn), casc(pn)

        Tt2 = np.asarray(-2.0 * T.T, F8)          # [d=128, m_global]
        stat8[b, :, :, 0, :] = Tt2.reshape(128, NT, 128)
        for j in range(3):
            stat8[b, j, :, 1, :] = one8                       # pn ones
            stat8[b, 3 + j, :, 1, :] = tn_t[j].reshape(NT, 128)

        mov8[b, :, 0, :] = np.asarray(P.T, F8)    # [d, n]
        for j in range(3):
            mov8[b, j, 1, :] = pn_t[j]
            mov8[b, 3 + j, 1, :] = one8
    return stat8, mov8


def run_device(pred, target, trace=False, **kw):
    from concourse.bass_utils import run_bass_kernel_spmd

    nc = _get_nc()
    ins = []
    for i in range(NCORES):
        sl = slice(i * BL, (i + 1) * BL)
        stat8, mov8 = _pack_core(pred[sl], target[sl])
        ins.append({"stat8": stat8, "mov8": mov8})
    return run_bass_kernel_spmd(nc, ins, list(range(NCORES)), trace=trace, **kw)


def kernel(pred, target):
    pred = np.ascontiguousarray(np.asarray(pred, dtype=np.float32))
    target = np.ascontiguousarray(np.asarray(target, dtype=np.float32))
    res = run_device(pred, target)
    rs = res.results

    # chamfer min over dim=0 (batch): cross-core elementwise min of acc
    d0 = rs[0]["ch0_part"].astype(np.float32)
    for r in rs[1:]:
        d0 = np.minimum(d0, r["ch0_part"].astype(np.float32))
    # col N-1 was overwritten by the scan output on device; recompute exact
    lastp = pred[:, N - 1, :]                              # [B, D]
    dlast = ((target.astype(np.float64)
              - lastp[:, None, :].astype(np.float64)) ** 2).sum(-1)  # [B, N]
    d0[:, N - 1] = dlast.min(axis=0)
    ch0 = np.sqrt(np.maximum(d0.astype(np.float64), 1e-12)).mean()

    # chamfer min over dim=1: scan cols, [core][p, b_local, mt] -> [B, N]
    ch1 = np.concatenate(
        [r["ch1_part"].astype(np.float64).transpose(1, 2, 0).reshape(BL, N)
         for r in rs], axis=0)                              # [B, N]
    ch1 = np.sqrt(np.maximum(ch1, 1e-12)).mean()

    mae = np.abs(pred.astype(np.float64) - target.astype(np.float64)).mean()

    p = np.sort(pred.reshape(B, -1), axis=1)
    g = np.sort(target.reshape(B, -1), axis=1)
    emd = np.abs(p - g).mean(axis=1, dtype=np.float64)

    return (mae + ch0 + ch1 + emd).astype(np.float32)
